# revision 27
# baseline (speedup 1.0000x reference)
"""DCNv3 forward on 8 trn2 NeuronCores — collapsed-tap formulation.

Strategy (data-parallel over (batch, H-half) -> 8 shards, as before), but
the per-pixel sampling sum is restructured from 321 (g,p,i,j) cells into
~110 per-group integer taps:

  out[q, g*32+c] = sum_{(ty,tx) in box_g} A_g[q,ty,tx] * Xpad[qy+1+ty, qx+1+tx, c]
  A_g[q,ty,tx]   = sum_p m[q,g,p] * hat(ry[q,g,p]-ty) * hat(rx[q,g,p]-tx)

with hat(d) = relu(1-|d|) evaluated on a fixed per-group integer tap grid
(absolute coords), so the collapse over p needs no scatter: hats are zero
outside each p's 2-wide support automatically.

Engine split per 8-row tile:
  PE    : offset/mask projections (f16 matmuls)
  ACT   : PSUM->SBUF copies, exp, and the A -> A-broadcast-over-c expansion
  DVE   : hat grids (tensor_scalar 4x f16), big tensor_tensor multiplies,
          in-place halving-tree tap reduction (2x f16)
  Pool  : per-p products + p-reduction of A
All hot ops are [128-partition x few-thousand-elem] f16 with packed
innermost dims, instead of the previous 20k tiny 32-elem fused ops.
"""

import numpy as np
import sys

sys.path.insert(0, "/opt/trn_rl_repo")

import concourse.bass as bass
import concourse.bacc as bacc
import concourse.mybir as mybir
import concourse.tile as tile
from concourse.bass_utils import run_bass_kernel_spmd

B, C, H, W = 4, 128, 128, 128
G, P, gc = 4, 9, 32
N_CORES = 8
HS = H // 2
RT = 8
NTILES = HS // RT
NT = 6                 # hat-grid taps per axis per group (padded)

f16 = mybir.dt.float16
f32 = mybir.dt.float32

_KS = np.array([-1.0, 0.0, 1.0], np.float32)
KX = np.repeat(_KS, 3)
KY = np.tile(_KS, 3)

# engine assignment knobs (tuned against TimelineSim)
ENG_PROD = "gpsimd"    # per-p products
ENG_RED = "gpsimd"     # p-reduction
ENG_HYM = "gpsimd"     # hy*m fold


class _Geom:
    pass


def _make_geom(inp, W_off, b_off):
    """Per-group tap boxes from the actual offset field."""
    xhw = inp.reshape(B, H, W, C)
    off = (xhw.reshape(-1, C) @ W_off + b_off).reshape(-1, G, P, 2)
    rx = off[..., 0] + KX
    ry = off[..., 1] + KY
    g = _Geom()
    g.txmin = np.floor(rx.min(axis=0)).min(axis=1).astype(np.int64)
    g.txmax = (np.floor(rx.max(axis=0)) + 1).max(axis=1).astype(np.int64)
    g.tymin = np.floor(ry.min(axis=0)).min(axis=1).astype(np.int64)
    g.tymax = (np.floor(ry.max(axis=0)) + 1).max(axis=1).astype(np.int64)
    g.nx = (g.txmax - g.txmin + 1).astype(np.int64)
    g.ny = (g.tymax - g.tymin + 1).astype(np.int64)
    assert g.nx.max() <= NT and g.ny.max() <= NT, (g.nx, g.ny)
    g.TXMIN = int(g.txmin.min()); g.TXMAX = int(g.txmax.max())
    g.TYMIN = int(g.tymin.min()); g.TYMAX = int(g.tymax.max())
    g.NS = g.TXMAX - g.TXMIN + 1
    g.NR = RT + g.TYMAX - g.TYMIN
    g.SR = HS + g.TYMAX - g.TYMIN
    g.SC = W + g.NS - 1
    g.ox = (g.txmin - g.TXMIN).astype(np.int64)
    g.oy = (g.tymin - g.TYMIN).astype(np.int64)
    return g


def _build(g: "_Geom"):
    nc = bacc.Bacc("TRN2", target_bir_lowering=False, debug=False,
                   num_devices=N_CORES)

    xslab_t = nc.dram_tensor("xslab", [g.SR * g.SC * C], f16, kind="ExternalInput")
    xchw_t = nc.dram_tensor("xchw", [C, HS * W], f16, kind="ExternalInput")
    wcat_t = nc.dram_tensor("wcat", [C, 108], f16, kind="ExternalInput")
    tgrid_t = nc.dram_tensor("tgrid", [C, 2 * NT * 36], f16, kind="ExternalInput")
    expb_t = nc.dram_tensor("expb", [C, 36], f16, kind="ExternalInput")
    pmask_t = nc.dram_tensor("pmask", [C, RT * NT * NT * P], f16, kind="ExternalInput")
    out_t = nc.dram_tensor("out", [HS * W * C], f32, kind="ExternalOutput")

    NS, NR = g.NS, g.NR
    mult, add, sub = mybir.AluOpType.mult, mybir.AluOpType.add, mybir.AluOpType.subtract
    amax, amin = mybir.AluOpType.abs_max, mybir.AluOpType.min
    AF = mybir.ActivationFunctionType

    def vap(v, off, dims):
        return bass.AP(tensor=v.tensor, offset=v.offset + off, ap=[v.ap[0]] + dims)

    lp = nc.allow_low_precision(reason="f16 tap sums; tolerance is 2e-2")
    lp.__enter__()

    with tile.TileContext(nc) as tc:
        with (
            tc.tile_pool(name="const", bufs=1) as cpool,
            tc.tile_pool(name="xs", bufs=2) as xspool,
            tc.tile_pool(name="work", bufs=2) as wpool,
            tc.tile_pool(name="early", bufs=2) as epool,
            tc.tile_pool(name="early1", bufs=1) as e1pool,
            tc.tile_pool(name="prodp", bufs=1) as ppool,
            tc.tile_pool(name="apply", bufs=2) as apool,
            tc.tile_pool(name="psum", bufs=2, space="PSUM") as pspool,
        ):
            wcat0 = cpool.tile([C, 108], f16)
            wcat = cpool.tile([C, 108], f16)
            tgrid = cpool.tile([C, 2 * NT * 36], f16)
            expb = cpool.tile([C, 36], f16)
            nc.sync.dma_start(wcat0[:], wcat_t.ap())
            nc.sync.dma_start(tgrid[:], tgrid_t.ap())
            nc.sync.dma_start(expb[:], expb_t.ap())
            # matmul deps must arrive on one sem -> stationary via ACT copy
            nc.scalar.copy(wcat[:], wcat0[:])

            E = {"vector": nc.vector, "gpsimd": nc.gpsimd}
            stash = {}

            def stage_a1(t):
                """Loads + projections for tile t (no ACT PSUM work yet)."""
                xs = xspool.tile([C, NS * NR * C], f16, name="xs")
                for si in range(NS):
                    src = bass.AP(
                        tensor=xslab_t,
                        offset=(RT * t * g.SC + si) * C,
                        ap=[[C, W], [g.SC * C, NR], [1, C]])
                    nc.sync.dma_start(
                        vap(xs[:], si * NR * C, [[C, NR], [1, C]]), src)

                xc0 = wpool.tile([C, RT * W], f16, name="xc0")
                nc.sync.dma_start(
                    xc0[:], bass.AP(tensor=xchw_t, offset=RT * t * W,
                                    ap=[[HS * W, C], [1, RT * W]]))
                xc = wpool.tile([C, RT * W], f16, name="xc")
                nc.scalar.copy(xc[:], xc0[:])

                # projections (PE); PSUM consumers stay on ACT so the
                # Matmult WAR deps arrive on a single sem
                praws = []
                for hh in range(2):
                    praw = pspool.tile([C, 4 * 108], f32, name=f"praw{hh}")
                    for k in range(4):
                        r = hh * 4 + k
                        nc.tensor.matmul(
                            vap(praw[:], k * 108, [[1, 108]]),
                            xc[:, r * W:(r + 1) * W], wcat[:],
                            start=True, stop=True)
                    praws.append(praw)
                stash[t] = (xs, praws)

            def stage_a2(t):
                """PSUM evacuation (ACT) for tile t."""
                xs, praws = stash.pop(t)
                rawq = wpool.tile([C, RT * 72], f16, name="rawq")
                el = wpool.tile([C, RT * 36], f16, name="el")
                for hh, praw in enumerate(praws):
                    nc.scalar.copy(
                        vap(rawq[:], hh * 4 * 72, [[1, 4 * 72]]),
                        vap(praw[:], 0, [[108, 4], [1, 72]]))
                    nc.scalar.activation(
                        vap(el[:], hh * 4 * 36, [[1, 4 * 36]]),
                        vap(praw[:], 72, [[108, 4], [1, 36]]), AF.Exp)
                stash[t] = (xs, rawq, el)

            # 0/1 mask for the segmented p-scan: 0 at p==0 resets the state
            pmask = cpool.tile([C, RT * NT * NT * P], f16)
            nc.sync.dma_start(pmask[:], pmask_t.ap())

            def front(t):
                """DVE front-end for tile t: softmax weights + hat grids."""
                xs, rawq, el = stash.pop(t)
                el2 = epool.tile([C, RT * 36], f16, name="el2")
                nc.vector.tensor_tensor(
                    vap(el2[:], 0, [[36, RT], [1, 36]]),
                    vap(el[:], 0, [[36, RT], [1, 36]]),
                    vap(expb[:], 0, [[0, RT], [1, 36]]), mult)
                den = epool.tile([C, RT * G], f32, name="den")
                nc.vector.tensor_reduce(
                    vap(den[:], 0, [[G, RT], [1, G]]),
                    vap(el2[:], 0, [[36, RT], [9, G], [1, P]]),
                    mybir.AxisListType.X, add)
                denr = epool.tile([C, RT * G], f32, name="denr")
                nc.vector.reciprocal(denr[:], den[:])
                mw = epool.tile([C, RT * 36], f16, name="mw")
                nc.vector.tensor_tensor(
                    vap(mw[:], 0, [[36, RT], [9, G], [1, P]]),
                    vap(el2[:], 0, [[36, RT], [9, G], [1, P]]),
                    vap(denr[:], 0, [[G, RT], [1, G], [0, P]]), mult)

                # d[r, ax, t, gp] = raw_offset - (tap - b_off - k)
                # (ISA mem patterns allow at most 3 free dims -> one op per axis)
                dmat = e1pool.tile([C, RT * 2 * NT * 36], f16, name="dmat")
                for ax in range(2):
                    nc.vector.tensor_tensor(
                        vap(dmat[:], ax * NT * 36,
                            [[2 * NT * 36, RT], [36, NT], [1, 36]]),
                        vap(rawq[:], ax * 36, [[72, RT], [0, NT], [1, 36]]),
                        vap(tgrid[:], ax * NT * 36, [[0, RT], [36, NT], [1, 36]]),
                        sub)
                # negated hats: hp = min(|d|,1) - 1 = -hat; the two negations
                # cancel in the products hym*hp_x (abs_max is not a valid HW
                # tensor_scalar op, so |d| comes from the ACT engine)
                m1 = e1pool.tile([C, RT * 2 * NT * 36], f16, name="m1")
                nc.scalar.activation(m1[:], dmat[:], AF.Abs)
                hp = epool.tile([C, RT * 2 * NT * 36], f16, name="hp")
                nc.vector.tensor_scalar(hp[:], m1[:], 1.0, 1.0, amin, sub)
                stash[t] = (xs, mw, hp)

            def front_pool(t):
                """hy*m fold for tile t (consumed by next tile's products)."""
                xs, mw, hp = stash.pop(t)
                hym = epool.tile([C, RT * NT * 36], f16, name="hym")
                E[ENG_HYM].tensor_tensor(
                    vap(hym[:], 0, [[NT * 36, RT], [36, NT], [1, 36]]),
                    vap(hp[:], 0, [[2 * NT * 36, RT], [36, NT], [1, 36]]),
                    vap(mw[:], 0, [[36, RT], [0, NT], [1, 36]]), mult)
                stash[t] = (xs, hp, hym)

            def apply_tile(t):
                """Per-group collapse + apply + output for tile t."""
                xs, hp, hym = stash.pop(t)
                acc = wpool.tile([C, RT * C], f32, name="acc")
                for gg in range(G):
                    ny, nx = int(g.ny[gg]), int(g.nx[gg])
                    bb = ny * nx
                    prod = ppool.tile([C, RT * bb * P], f16, name=f"prod{gg}")
                    for ty in range(ny):
                        E[ENG_PROD].tensor_tensor(
                            vap(prod[:], ty * nx * P,
                                [[bb * P, RT], [P, nx], [1, P]]),
                            vap(hym[:], gg * P + ty * 36,
                                [[NT * 36, RT], [0, nx], [1, P]]),
                            vap(hp[:], NT * 36 + gg * P,
                                [[2 * NT * 36, RT], [36, nx], [1, P]]),
                            mult)
                    # p-reduction: in-place halving tree along the p axis
                    # (the scan opcode is not supported on Pool in HW codegen)
                    b = P
                    while b > 2:
                        h = (b + 1) // 2
                        E[ENG_RED].tensor_tensor(
                            vap(prod[:], 0, [[bb * P, RT], [P, bb], [1, b - h]]),
                            vap(prod[:], 0, [[bb * P, RT], [P, bb], [1, b - h]]),
                            vap(prod[:], h, [[bb * P, RT], [P, bb], [1, b - h]]),
                            add)
                        b = h
                    A = epool.tile([C, RT * bb], f16, name=f"A{gg}")
                    E[ENG_RED].tensor_tensor(
                        vap(A[:], 0, [[bb, RT], [1, bb]]),
                        vap(prod[:], 0, [[bb * P, RT], [P, bb]]),
                        vap(prod[:], 1, [[bb * P, RT], [P, bb]]),
                        add)
                    # broadcast A over the 32 channels (ACT, flat-rate)
                    cf = apool.tile([C, RT * bb * gc], f16, name="cf")
                    nc.scalar.copy(
                        vap(cf[:], 0, [[bb * gc, RT], [gc, bb], [1, gc]]),
                        vap(A[:], 0, [[bb, RT], [1, bb], [0, gc]]))
                    # tmp = X * cf   (all f16 packed -> 2x), one op per dy row
                    tmp = apool.tile([C, RT * bb * gc], f16, name="tmp")
                    xoff = int(g.ox[gg]) * NR * C + int(g.oy[gg]) * C + gg * gc
                    for ty in range(ny):
                        nc.vector.tensor_tensor(
                            vap(tmp[:], ty * nx * gc,
                                [[bb * gc, RT], [gc, nx], [1, gc]]),
                            vap(xs[:], xoff + ty * C,
                                [[C, RT], [NR * C, nx], [1, gc]]),
                            vap(cf[:], ty * nx * gc,
                                [[bb * gc, RT], [gc, nx], [1, gc]]),
                            mult)
                    # in-place halving tree over the bb taps
                    b = bb
                    while b > 2:
                        h = (b + 1) // 2
                        nc.vector.tensor_tensor(
                            vap(tmp[:], 0, [[bb * gc, RT], [gc, b - h], [1, gc]]),
                            vap(tmp[:], 0, [[bb * gc, RT], [gc, b - h], [1, gc]]),
                            vap(tmp[:], h * gc, [[bb * gc, RT], [gc, b - h], [1, gc]]),
                            add)
                        b = h
                    nc.vector.tensor_tensor(
                        vap(acc[:], gg * gc, [[C, RT], [1, gc]]),
                        vap(tmp[:], 0, [[bb * gc, RT], [1, gc]]),
                        vap(tmp[:], gc, [[bb * gc, RT], [1, gc]]),
                        add)
                return acc

            # software pipeline: DVE front-end of tile t+1 is queued before
            # tile t's apply so Pool/ACT of t run concurrently with DVE of t+1
            stage_a1(0)
            stage_a2(0)
            front(0)
            front_pool(0)
            for t in range(NTILES):
                if t + 1 < NTILES:
                    stage_a1(t + 1)
                    stage_a2(t + 1)
                    front(t + 1)
                acc = apply_tile(t)
                if t + 1 < NTILES:
                    front_pool(t + 1)
                # emitted late so its wait-for-acc doesn't delay the next
                # tile's loads in the serial SP DMA-issue stream
                nc.sync.dma_start(
                    bass.AP(tensor=out_t, offset=RT * t * W * C,
                            ap=[[C, W], [W * C, RT], [1, C]]),
                    vap(acc[:], 0, [[C, RT], [1, C]]))

    lp.__exit__(None, None, None)
    nc.compile()
    return nc


def _host_prep(inp, W_off, b_off, W_mask, b_mask, g):
    xhw = inp.reshape(B, H, W, C)

    wcat = np.empty((C, 108), np.float32)
    for gg in range(G):
        for p in range(P):
            gp = gg * P + p
            wcat[:, gp] = W_off[:, 2 * gp + 1]           # y
            wcat[:, 36 + gp] = W_off[:, 2 * gp]          # x
            wcat[:, 72 + gp] = W_mask[:, gp]
    wcat16 = wcat.astype(np.float16)

    # tgrid[ax, t, gp] = tap - b_off - k   (so d = raw - tgrid = r - tap)
    tg = np.empty((2, NT, 36), np.float32)
    for gg in range(G):
        for p in range(P):
            gp = gg * P + p
            for t in range(NT):
                tg[0, t, gp] = (g.tymin[gg] + t) - b_off[2 * gp + 1] - KY[p]
                tg[1, t, gp] = (g.txmin[gg] + t) - b_off[2 * gp] - KX[p]
    tgrid = np.tile(tg.reshape(1, -1), (C, 1)).astype(np.float16)
    expb = np.tile(np.exp(b_mask)[None, :].astype(np.float16), (C, 1))
    pm = (np.arange(RT * NT * NT * P) % P != 0).astype(np.float16)
    pmask = np.tile(pm[None, :], (C, 1))

    x16 = xhw.astype(np.float16)
    in_maps = []
    for core in range(N_CORES):
        b, half = divmod(core, 2)
        h0 = HS * half
        E = np.zeros((H + 8, W + 8, C), np.float16)
        E[4:4 + H, 4:4 + W] = x16[b]
        slab = E[4 + h0 + g.TYMIN: 4 + h0 + g.TYMIN + g.SR,
                 4 + g.TXMIN: 4 + g.TXMIN + g.SC]
        xchw = np.ascontiguousarray(
            x16[b, h0:h0 + HS].reshape(HS * W, C).T)
        in_maps.append({
            "xslab": np.ascontiguousarray(slab).reshape(-1),
            "xchw": xchw,
            "wcat": wcat16,
            "tgrid": tgrid,
            "expb": expb,
            "pmask": pmask,
        })
    return in_maps


def _run(inp, W_off, b_off, W_mask, b_mask, **spmd_kwargs):
    inp = np.ascontiguousarray(inp, np.float32)
    W_off = np.asarray(W_off, np.float32)
    b_off = np.asarray(b_off, np.float32)
    g = _make_geom(inp, W_off, b_off)
    nc = _build(g)
    in_maps = _host_prep(inp, W_off, b_off,
                         np.asarray(W_mask, np.float32),
                         np.asarray(b_mask, np.float32), g)
    res = run_bass_kernel_spmd(nc, in_maps, core_ids=list(range(N_CORES)),
                               **spmd_kwargs)
    out = np.empty((B, H, W, C), np.float32)
    for core in range(N_CORES):
        b, half = divmod(core, 2)
        out[b, HS * half:HS * (half + 1)] = \
            res.results[core]["out"].reshape(HS, W, C)
    return out.reshape(B, C, H, W), res


def kernel(inp, W_off, b_off, W_mask, b_mask):
    out, _ = _run(inp, W_off, b_off, W_mask, b_mask)
    return out


if __name__ == "__main__":
    d = np.load("/root/problem/ref_cache.npz")
    got = kernel(d["inp"], d["W_off"], d["b_off"], d["W_mask"], d["b_mask"])
    exp = d["exp"]
    err = np.abs(got - exp).max()
    print("absmax err:", err, "rel:", err / np.abs(exp).max())


# revision 30
# speedup vs baseline: 100.8532x; 100.8532x over previous
"""DCNv3 forward on 8 trn2 NeuronCores — collapsed-tap formulation.

Strategy (data-parallel over (batch, H-half) -> 8 shards, as before), but
the per-pixel sampling sum is restructured from 321 (g,p,i,j) cells into
~110 per-group integer taps:

  out[q, g*32+c] = sum_{(ty,tx) in box_g} A_g[q,ty,tx] * Xpad[qy+1+ty, qx+1+tx, c]
  A_g[q,ty,tx]   = sum_p m[q,g,p] * hat(ry[q,g,p]-ty) * hat(rx[q,g,p]-tx)

with hat(d) = relu(1-|d|) evaluated on a fixed per-group integer tap grid
(absolute coords), so the collapse over p needs no scatter: hats are zero
outside each p's 2-wide support automatically.

Engine split per 8-row tile:
  PE    : offset/mask projections (f16 matmuls)
  ACT   : PSUM->SBUF copies, exp, and the A -> A-broadcast-over-c expansion
  DVE   : hat grids (tensor_scalar 4x f16), big tensor_tensor multiplies,
          in-place halving-tree tap reduction (2x f16)
  Pool  : per-p products + p-reduction of A
All hot ops are [128-partition x few-thousand-elem] f16 with packed
innermost dims, instead of the previous 20k tiny 32-elem fused ops.
"""

import numpy as np
import sys

sys.path.insert(0, "/opt/trn_rl_repo")

import concourse.bass as bass
import concourse.bacc as bacc
import concourse.mybir as mybir
import concourse.tile as tile
from concourse.bass_utils import run_bass_kernel_spmd

B, C, H, W = 4, 128, 128, 128
G, P, gc = 4, 9, 32
N_CORES = 8
HS = H // 2
RT = 8
NTILES = HS // RT
NT = 6                 # hat-grid taps per axis per group (padded)

f16 = mybir.dt.float16
f32 = mybir.dt.float32

_KS = np.array([-1.0, 0.0, 1.0], np.float32)
KX = np.repeat(_KS, 3)
KY = np.tile(_KS, 3)

# engine assignment knobs (tuned against TimelineSim)
ENG_PROD = "gpsimd"    # per-p products
ENG_RED = "gpsimd"     # p-reduction
ENG_HYM = "gpsimd"     # hy*m fold
ENG_ARED = "gpsimd"    # final A extraction add


class _Geom:
    pass


def _make_geom(inp, W_off, b_off):
    """Per-group tap boxes from the actual offset field."""
    xhw = inp.reshape(B, H, W, C)
    off = (xhw.reshape(-1, C) @ W_off + b_off).reshape(-1, G, P, 2)
    rx = off[..., 0] + KX
    ry = off[..., 1] + KY
    g = _Geom()
    g.txmin = np.floor(rx.min(axis=0)).min(axis=1).astype(np.int64)
    g.txmax = (np.floor(rx.max(axis=0)) + 1).max(axis=1).astype(np.int64)
    g.tymin = np.floor(ry.min(axis=0)).min(axis=1).astype(np.int64)
    g.tymax = (np.floor(ry.max(axis=0)) + 1).max(axis=1).astype(np.int64)
    g.nx = (g.txmax - g.txmin + 1).astype(np.int64)
    g.ny = (g.tymax - g.tymin + 1).astype(np.int64)
    assert g.nx.max() <= NT and g.ny.max() <= NT, (g.nx, g.ny)
    g.TXMIN = int(g.txmin.min()); g.TXMAX = int(g.txmax.max())
    g.TYMIN = int(g.tymin.min()); g.TYMAX = int(g.tymax.max())
    g.NS = g.TXMAX - g.TXMIN + 1
    g.NR = RT + g.TYMAX - g.TYMIN
    g.SR = HS + g.TYMAX - g.TYMIN
    g.SC = W + g.NS - 1
    g.ox = (g.txmin - g.TXMIN).astype(np.int64)
    g.oy = (g.tymin - g.TYMIN).astype(np.int64)
    return g


def _build(g: "_Geom", reps=1):
    nc = bacc.Bacc("TRN2", target_bir_lowering=False, debug=False,
                   num_devices=N_CORES)

    xslab_t = nc.dram_tensor("xslab", [g.SR * g.SC * C], f16, kind="ExternalInput")
    xchw_t = nc.dram_tensor("xchw", [C, HS * W], f16, kind="ExternalInput")
    wcat_t = nc.dram_tensor("wcat", [C, 108], f16, kind="ExternalInput")
    tgrid_t = nc.dram_tensor("tgrid", [C, 2 * NT * 36], f16, kind="ExternalInput")
    expb_t = nc.dram_tensor("expb", [C, 36], f16, kind="ExternalInput")
    pmask_t = nc.dram_tensor("pmask", [C, RT * NT * NT * P], f16, kind="ExternalInput")
    out_t = nc.dram_tensor("out", [HS * W * C], f32, kind="ExternalOutput")

    NS, NR = g.NS, g.NR
    mult, add, sub = mybir.AluOpType.mult, mybir.AluOpType.add, mybir.AluOpType.subtract
    amax, amin = mybir.AluOpType.abs_max, mybir.AluOpType.min
    AF = mybir.ActivationFunctionType

    def vap(v, off, dims):
        return bass.AP(tensor=v.tensor, offset=v.offset + off, ap=[v.ap[0]] + dims)

    lp = nc.allow_low_precision(reason="f16 tap sums; tolerance is 2e-2")
    lp.__enter__()

    with tile.TileContext(nc) as tc:
        with (
            tc.tile_pool(name="const", bufs=1) as cpool,
            tc.tile_pool(name="xs", bufs=2) as xspool,
            tc.tile_pool(name="work", bufs=2) as wpool,
            tc.tile_pool(name="early", bufs=2) as epool,
            tc.tile_pool(name="early1", bufs=1) as e1pool,
            tc.tile_pool(name="prodp", bufs=1) as ppool,
            tc.tile_pool(name="apply", bufs=2) as apool,
            tc.tile_pool(name="psum", bufs=2, space="PSUM") as pspool,
        ):
            wcat0 = cpool.tile([C, 108], f16)
            wcat = cpool.tile([C, 108], f16)
            tgrid = cpool.tile([C, 2 * NT * 36], f16)
            expb = cpool.tile([C, 36], f16)
            nc.sync.dma_start(wcat0[:], wcat_t.ap())
            nc.sync.dma_start(tgrid[:], tgrid_t.ap())
            nc.sync.dma_start(expb[:], expb_t.ap())
            # matmul deps must arrive on one sem -> stationary via ACT copy
            nc.scalar.copy(wcat[:], wcat0[:])

            E = {"vector": nc.vector, "gpsimd": nc.gpsimd}
            stash = {}

            def stage_a1(t):
                """Loads + projections for tile t (no ACT PSUM work yet)."""
                xs = xspool.tile([C, NS * NR * C], f16, name="xs")
                tt = t % NTILES
                for si in range(NS):
                    src = bass.AP(
                        tensor=xslab_t,
                        offset=(RT * tt * g.SC + si) * C,
                        ap=[[C, W], [g.SC * C, NR], [1, C]])
                    nc.sync.dma_start(
                        vap(xs[:], si * NR * C, [[C, NR], [1, C]]), src)

                xc0 = wpool.tile([C, RT * W], f16, name="xc0")
                nc.sync.dma_start(
                    xc0[:], bass.AP(tensor=xchw_t, offset=RT * tt * W,
                                    ap=[[HS * W, C], [1, RT * W]]))
                xc = wpool.tile([C, RT * W], f16, name="xc")
                nc.scalar.copy(xc[:], xc0[:])

                # projections (PE); PSUM consumers stay on ACT so the
                # Matmult WAR deps arrive on a single sem
                praws = []
                for hh in range(2):
                    praw = pspool.tile([C, 4 * 108], f32, name=f"praw{hh}")
                    for k in range(4):
                        r = hh * 4 + k
                        nc.tensor.matmul(
                            vap(praw[:], k * 108, [[1, 108]]),
                            xc[:, r * W:(r + 1) * W], wcat[:],
                            start=True, stop=True)
                    praws.append(praw)
                stash[t] = (xs, praws)

            def stage_a2(t):
                """PSUM evacuation (ACT) for tile t."""
                xs, praws = stash.pop(t)
                rawq = wpool.tile([C, RT * 72], f16, name="rawq")
                el = wpool.tile([C, RT * 36], f16, name="el")
                for hh, praw in enumerate(praws):
                    nc.scalar.copy(
                        vap(rawq[:], hh * 4 * 72, [[1, 4 * 72]]),
                        vap(praw[:], 0, [[108, 4], [1, 72]]))
                    nc.scalar.activation(
                        vap(el[:], hh * 4 * 36, [[1, 4 * 36]]),
                        vap(praw[:], 72, [[108, 4], [1, 36]]), AF.Exp)
                stash[t] = (xs, rawq, el)

            # 0/1 mask for the segmented p-scan: 0 at p==0 resets the state
            pmask = cpool.tile([C, RT * NT * NT * P], f16)
            nc.sync.dma_start(pmask[:], pmask_t.ap())

            def front(t):
                """DVE front-end for tile t: softmax weights + hat grids."""
                xs, rawq, el = stash.pop(t)
                el2 = epool.tile([C, RT * 36], f16, name="el2")
                nc.vector.tensor_tensor(
                    vap(el2[:], 0, [[36, RT], [1, 36]]),
                    vap(el[:], 0, [[36, RT], [1, 36]]),
                    vap(expb[:], 0, [[0, RT], [1, 36]]), mult)
                den = epool.tile([C, RT * G], f32, name="den")
                nc.vector.tensor_reduce(
                    vap(den[:], 0, [[G, RT], [1, G]]),
                    vap(el2[:], 0, [[36, RT], [9, G], [1, P]]),
                    mybir.AxisListType.X, add)
                denr = epool.tile([C, RT * G], f32, name="denr")
                nc.vector.reciprocal(denr[:], den[:])
                mw = epool.tile([C, RT * 36], f16, name="mw")
                nc.vector.tensor_tensor(
                    vap(mw[:], 0, [[36, RT], [9, G], [1, P]]),
                    vap(el2[:], 0, [[36, RT], [9, G], [1, P]]),
                    vap(denr[:], 0, [[G, RT], [1, G], [0, P]]), mult)

                # d[r, ax, t, gp] = raw_offset - (tap - b_off - k)
                # (ISA mem patterns allow at most 3 free dims -> one op per axis)
                dmat = e1pool.tile([C, RT * 2 * NT * 36], f16, name="dmat")
                for ax in range(2):
                    nc.vector.tensor_tensor(
                        vap(dmat[:], ax * NT * 36,
                            [[2 * NT * 36, RT], [36, NT], [1, 36]]),
                        vap(rawq[:], ax * 36, [[72, RT], [0, NT], [1, 36]]),
                        vap(tgrid[:], ax * NT * 36, [[0, RT], [36, NT], [1, 36]]),
                        sub)
                # negated hats: hp = min(|d|,1) - 1 = -hat; the two negations
                # cancel in the products hym*hp_x (abs_max is not a valid HW
                # tensor_scalar op, so |d| comes from the ACT engine)
                m1 = e1pool.tile([C, RT * 2 * NT * 36], f16, name="m1")
                nc.scalar.activation(m1[:], dmat[:], AF.Abs)
                hp = epool.tile([C, RT * 2 * NT * 36], f16, name="hp")
                nc.vector.tensor_scalar(hp[:], m1[:], 1.0, 1.0, amin, sub)
                stash[t] = (xs, mw, hp)

            def front_pool(t):
                """hy*m fold for tile t (consumed by next tile's products)."""
                xs, mw, hp = stash.pop(t)
                hym = epool.tile([C, RT * NT * 36], f16, name="hym")
                E[ENG_HYM].tensor_tensor(
                    vap(hym[:], 0, [[NT * 36, RT], [36, NT], [1, 36]]),
                    vap(hp[:], 0, [[2 * NT * 36, RT], [36, NT], [1, 36]]),
                    vap(mw[:], 0, [[36, RT], [0, NT], [1, 36]]), mult)
                stash[t] = (xs, hp, hym)

            def apply_tile(t):
                """Per-group collapse + apply + output for tile t."""
                xs, hp, hym = stash.pop(t)
                acc = wpool.tile([C, RT * C], f32, name="acc")
                for gg in range(G):
                    ny, nx = int(g.ny[gg]), int(g.nx[gg])
                    bb = ny * nx
                    prod = ppool.tile([C, RT * bb * P], f16, name=f"prod{gg}")
                    for ty in range(ny):
                        E[ENG_PROD].tensor_tensor(
                            vap(prod[:], ty * nx * P,
                                [[bb * P, RT], [P, nx], [1, P]]),
                            vap(hym[:], gg * P + ty * 36,
                                [[NT * 36, RT], [0, nx], [1, P]]),
                            vap(hp[:], NT * 36 + gg * P,
                                [[2 * NT * 36, RT], [36, nx], [1, P]]),
                            mult)
                    # p-reduction: in-place halving tree along the p axis
                    # (the scan opcode is not supported on Pool in HW codegen)
                    b = P
                    while b > 2:
                        h = (b + 1) // 2
                        E[ENG_RED].tensor_tensor(
                            vap(prod[:], 0, [[bb * P, RT], [P, bb], [1, b - h]]),
                            vap(prod[:], 0, [[bb * P, RT], [P, bb], [1, b - h]]),
                            vap(prod[:], h, [[bb * P, RT], [P, bb], [1, b - h]]),
                            add)
                        b = h
                    A = epool.tile([C, RT * bb], f16, name=f"A{gg}")
                    E[ENG_ARED].tensor_tensor(
                        vap(A[:], 0, [[bb, RT], [1, bb]]),
                        vap(prod[:], 0, [[bb * P, RT], [P, bb]]),
                        vap(prod[:], 1, [[bb * P, RT], [P, bb]]),
                        add)
                    # broadcast A over the 32 channels (ACT, flat-rate)
                    cf = apool.tile([C, RT * bb * gc], f16, name="cf")
                    nc.scalar.copy(
                        vap(cf[:], 0, [[bb * gc, RT], [gc, bb], [1, gc]]),
                        vap(A[:], 0, [[bb, RT], [1, bb], [0, gc]]))
                    # tmp = X * cf   (all f16 packed -> 2x), one op per dy row
                    tmp = apool.tile([C, RT * bb * gc], f16, name="tmp")
                    xoff = int(g.ox[gg]) * NR * C + int(g.oy[gg]) * C + gg * gc
                    for ty in range(ny):
                        nc.vector.tensor_tensor(
                            vap(tmp[:], ty * nx * gc,
                                [[bb * gc, RT], [gc, nx], [1, gc]]),
                            vap(xs[:], xoff + ty * C,
                                [[C, RT], [NR * C, nx], [1, gc]]),
                            vap(cf[:], ty * nx * gc,
                                [[bb * gc, RT], [gc, nx], [1, gc]]),
                            mult)
                    # in-place halving tree over the bb taps
                    b = bb
                    while b > 2:
                        h = (b + 1) // 2
                        nc.vector.tensor_tensor(
                            vap(tmp[:], 0, [[bb * gc, RT], [gc, b - h], [1, gc]]),
                            vap(tmp[:], 0, [[bb * gc, RT], [gc, b - h], [1, gc]]),
                            vap(tmp[:], h * gc, [[bb * gc, RT], [gc, b - h], [1, gc]]),
                            add)
                        b = h
                    nc.vector.tensor_tensor(
                        vap(acc[:], gg * gc, [[C, RT], [1, gc]]),
                        vap(tmp[:], 0, [[bb * gc, RT], [1, gc]]),
                        vap(tmp[:], gc, [[bb * gc, RT], [1, gc]]),
                        add)
                return acc

            # software pipeline: DVE front-end of tile t+1 is queued before
            # tile t's apply so Pool/ACT of t run concurrently with DVE of t+1
            # (reps>1 replicates the whole program for loop-in-kernel timing)
            NV = NTILES * reps
            stage_a1(0)
            stage_a2(0)
            front(0)
            front_pool(0)
            for vt in range(NV):
                if vt + 1 < NV:
                    stage_a1(vt + 1)
                    stage_a2(vt + 1)
                    front(vt + 1)
                acc = apply_tile(vt)
                if vt + 1 < NV:
                    front_pool(vt + 1)
                # emitted late so its wait-for-acc doesn't delay the next
                # tile's loads in the serial SP DMA-issue stream
                nc.sync.dma_start(
                    bass.AP(tensor=out_t, offset=RT * (vt % NTILES) * W * C,
                            ap=[[C, W], [W * C, RT], [1, C]]),
                    vap(acc[:], 0, [[C, RT], [1, C]]))

    lp.__exit__(None, None, None)
    nc.compile()
    return nc


def _host_prep(inp, W_off, b_off, W_mask, b_mask, g):
    xhw = inp.reshape(B, H, W, C)

    wcat = np.empty((C, 108), np.float32)
    for gg in range(G):
        for p in range(P):
            gp = gg * P + p
            wcat[:, gp] = W_off[:, 2 * gp + 1]           # y
            wcat[:, 36 + gp] = W_off[:, 2 * gp]          # x
            wcat[:, 72 + gp] = W_mask[:, gp]
    wcat16 = wcat.astype(np.float16)

    # tgrid[ax, t, gp] = tap - b_off - k   (so d = raw - tgrid = r - tap)
    tg = np.empty((2, NT, 36), np.float32)
    for gg in range(G):
        for p in range(P):
            gp = gg * P + p
            for t in range(NT):
                tg[0, t, gp] = (g.tymin[gg] + t) - b_off[2 * gp + 1] - KY[p]
                tg[1, t, gp] = (g.txmin[gg] + t) - b_off[2 * gp] - KX[p]
    tgrid = np.tile(tg.reshape(1, -1), (C, 1)).astype(np.float16)
    expb = np.tile(np.exp(b_mask)[None, :].astype(np.float16), (C, 1))
    pm = (np.arange(RT * NT * NT * P) % P != 0).astype(np.float16)
    pmask = np.tile(pm[None, :], (C, 1))

    x16 = xhw.astype(np.float16)
    in_maps = []
    for core in range(N_CORES):
        b, half = divmod(core, 2)
        h0 = HS * half
        E = np.zeros((H + 8, W + 8, C), np.float16)
        E[4:4 + H, 4:4 + W] = x16[b]
        slab = E[4 + h0 + g.TYMIN: 4 + h0 + g.TYMIN + g.SR,
                 4 + g.TXMIN: 4 + g.TXMIN + g.SC]
        xchw = np.ascontiguousarray(
            x16[b, h0:h0 + HS].reshape(HS * W, C).T)
        in_maps.append({
            "xslab": np.ascontiguousarray(slab).reshape(-1),
            "xchw": xchw,
            "wcat": wcat16,
            "tgrid": tgrid,
            "expb": expb,
            "pmask": pmask,
        })
    return in_maps


def _run(inp, W_off, b_off, W_mask, b_mask, **spmd_kwargs):
    inp = np.ascontiguousarray(inp, np.float32)
    W_off = np.asarray(W_off, np.float32)
    b_off = np.asarray(b_off, np.float32)
    g = _make_geom(inp, W_off, b_off)
    nc = _build(g)
    in_maps = _host_prep(inp, W_off, b_off,
                         np.asarray(W_mask, np.float32),
                         np.asarray(b_mask, np.float32), g)
    res = run_bass_kernel_spmd(nc, in_maps, core_ids=list(range(N_CORES)),
                               **spmd_kwargs)
    out = np.empty((B, H, W, C), np.float32)
    for core in range(N_CORES):
        b, half = divmod(core, 2)
        out[b, HS * half:HS * (half + 1)] = \
            res.results[core]["out"].reshape(HS, W, C)
    return out.reshape(B, C, H, W), res


def kernel(inp, W_off, b_off, W_mask, b_mask):
    out, _ = _run(inp, W_off, b_off, W_mask, b_mask)
    return out


if __name__ == "__main__":
    d = np.load("/root/problem/ref_cache.npz")
    got = kernel(d["inp"], d["W_off"], d["b_off"], d["W_mask"], d["b_mask"])
    exp = d["exp"]
    err = np.abs(got - exp).max()
    print("absmax err:", err, "rel:", err / np.abs(exp).max())
